# revision 28
# baseline (speedup 1.0000x reference)
"""Trainium2 Bass kernel for hetero-GNN (2x ResGatedGraphConv + segment-mean pooling + MLP).

v3:
  - Host-streamed fp8 one-hot scatter matrices (no on-device IS_EQ; fp8 FWL LDW).
  - fp8 edge stream / weights / skip inputs; fp16 sigmoid output; fp8 messages.
  - Global dst-buckets sorted by edge count and dealt round-robin to (core, k)
    so the shared SPMD program's per-k subtile count (max over cores) is tight.
  - Sigmoid/multiply batched over GRP=8 subtiles in flat groups; every 3rd
    multiply offloaded to GPSIMD.
  - Skip-connection matmul opens each bucket's PSUM accumulation; ReLU fused
    into the PSUM->SBUF copy; pooling matmul emitted inline per bucket.
"""
import sys
import types
import numpy as np
import ml_dtypes

NCORES = 8
G = 128
H = 64
F = 16
NC_N = 100000
NB_N = 200000
BUCKET = 128
GRP = 6  # subtiles (x128 edges) per streaming group; must be multiple of 3
FP8 = True
GP_EVERY = 0  # gpsimd multiply offload disabled: GPSIMD cannot access PSUM
LAST_EXEC_NS = None

F8NP = np.dtype(ml_dtypes.float8_e4m3)
ONE_F8 = np.float32(1.0).astype(F8NP).view(np.uint8)
EDT = F8NP if FP8 else np.float16


def _install_ntff_shim():
    if 'antenv.axon_hooks' in sys.modules:
        return
    try:
        mod = types.ModuleType('antenv.axon_hooks')
        _h = [None]
        mod.set_axon_ntff_profile_hook = lambda h: _h.__setitem__(0, h)
        mod.get_axon_ntff_profile_hook = lambda: _h[0]
        sys.modules['antenv.axon_hooks'] = mod
        import antenv
        antenv.axon_hooks = mod
        from trn_agent_boot.trn_boot import _ntff_profile_via_ctypes
        mod.set_axon_ntff_profile_hook(
            _ntff_profile_via_ctypes('/opt/axon/libaxon_pjrt.so'))
    except Exception:
        pass


def _prep_relation(x_src, x_dst, src, dst, ea, batch, n_dst):
    """Host marshalling with bucket dealing.

    Global 128-dst buckets are sorted by edge count (desc) and dealt to
    (core, k): core m's k-th bucket is sorted_buckets[8k+m]. All cores share
    the same per-k subtile count = ceil(largest bucket of round k / 128).
    Returns per-core xt/oh streams plus pa (skip features), bt (graph ids).
    """
    nbuck_g = (n_dst + BUCKET - 1) // BUCKET
    nbuck = (nbuck_g + NCORES - 1) // NCORES
    n_slots_g = nbuck * NCORES  # padded with empty buckets

    gb = dst // BUCKET
    cnt = np.bincount(gb, minlength=n_slots_g)
    order_b = np.argsort(-cnt, kind="stable")  # bucket ids, desc by count
    # round k takes sorted buckets [8k, 8k+8); per-k subtiles from the max
    subtiles = np.maximum(
        (cnt[order_b[::NCORES]] + 127) // 128, 1)  # [nbuck]
    ntiles = int(subtiles.sum())
    ntiles3 = ((ntiles + 2) // 3) * 3  # transform packs 3 subtiles per matmul
    ntot = ntiles3 * 128
    starts = np.zeros(nbuck + 1, np.int64)
    starts[1:] = np.cumsum(subtiles) * 128

    # bucket -> (core, k)
    core_of_b = np.empty(n_slots_g, np.int64)
    k_of_b = np.empty(n_slots_g, np.int64)
    core_of_b[order_b] = np.arange(n_slots_g) % NCORES
    k_of_b[order_b] = np.arange(n_slots_g) // NCORES

    edge_core = core_of_b[gb]
    edge_k = k_of_b[gb]
    xs = x_src.astype(EDT)
    xd = x_dst.astype(EDT)
    batf = batch.astype(np.float32)

    per_core = []
    for m in range(NCORES):
        sel = np.nonzero(edge_core == m)[0]
        # sort this core's edges by k (stable)
        sel = sel[np.argsort(edge_k[sel], kind="stable")]
        c_src, c_dst, c_ea = src[sel], dst[sel], ea[sel, 0]
        c_k = edge_k[sel]
        pos_in_bucket = np.arange(len(sel)) - np.searchsorted(c_k, c_k)
        slot = starts[c_k] + pos_in_bucket
        xt = np.zeros((34, ntot), EDT)
        xt[0:16, slot] = xs[c_src].T
        xt[16, slot] = c_ea.astype(EDT)
        xt[17, slot] = np.float32(1.0)
        xt[18:34, slot] = xd[c_dst].T
        # stack 3 consecutive subtiles along K: xt3[:, t*128+m] holds the
        # 34-row features of subtiles 3t, 3t+1, 3t+2 at slot m
        xtr = xt.reshape(34, ntiles3, 128)
        xt3 = np.concatenate([xtr[:, 0::3], xtr[:, 1::3], xtr[:, 2::3]],
                             axis=0).reshape(102, (ntiles3 // 3) * 128)
        xt3 = np.ascontiguousarray(xt3)
        oh = np.zeros((128, ntot), np.uint8)
        d_loc = c_dst % BUCKET
        oh[slot % 128, (slot // 128) * 128 + d_loc] = ONE_F8
        # skip features + streamed pooling one-hot for this core's buckets
        pa = np.zeros((17, nbuck * BUCKET), EDT)
        og = np.zeros((128, nbuck * 128), np.uint8)
        my_b = order_b[np.arange(nbuck) * NCORES + m]  # global bucket per k
        for k in range(nbuck):
            b0 = my_b[k] * BUCKET
            w = min(BUCKET, n_dst - b0)
            if w <= 0:
                continue
            pa[0:16, k * BUCKET:k * BUCKET + w] = xd[b0:b0 + w].T
            pa[16, k * BUCKET:k * BUCKET + w] = np.float32(1.0)
            gids = batf[b0:b0 + w].astype(np.int64)
            og[np.arange(w), k * 128 + gids] = ONE_F8
        per_core.append({"xt": xt3, "oh": oh.view(F8NP), "pa": pa,
                         "og": og.view(F8NP)})
    return {"nbuck": nbuck, "subtiles": subtiles, "ntot": ntot,
            "ntiles3": ntiles3, "per_core": per_core}


def kernel(**inputs):
    _install_ntff_shim()
    import concourse.bass as bass  # noqa: F401
    import concourse.bacc as bacc
    import concourse.mybir as mybir
    import concourse.tile as tile
    from concourse.bass_utils import run_bass_kernel_spmd

    F32 = mybir.dt.float32
    F16 = mybir.dt.float16
    FE = mybir.dt.float8e4 if FP8 else mybir.dt.float16
    AF = mybir.ActivationFunctionType
    OP = mybir.AluOpType

    ii = {k: np.asarray(v) for k, v in inputs.items()}

    rel_c = _prep_relation(ii["x_x"], ii["x_c"], ii["src_ac"].astype(np.int64),
                           ii["dst_ac"].astype(np.int64), ii["ea_ac"],
                           ii["batch_c"].astype(np.int64), NC_N)
    rel_b = _prep_relation(ii["x_c"], ii["x_b"], ii["src_cb"].astype(np.int64),
                           ii["dst_cb"].astype(np.int64), ii["ea_cb"],
                           ii["batch_b"].astype(np.int64), NB_N)

    cnt_c = np.bincount(ii["batch_c"].astype(np.int64), minlength=G).astype(np.float32)
    cnt_b = np.bincount(ii["batch_b"].astype(np.int64), minlength=G).astype(np.float32)
    recip = np.stack([1.0 / np.maximum(cnt_c, 1.0),
                      1.0 / np.maximum(cnt_b, 1.0)]).astype(np.float16)  # [2, G]

    def waug(rel):
        Wq, Wv, Wk = ii[f"Wq_{rel}"], ii[f"Wv_{rel}"], ii[f"Wk_{rel}"]
        We = ii[f"We_{rel}"][0]
        bq, bv, bk, be = (ii[f"bq_{rel}"], ii[f"bv_{rel}"],
                          ii[f"bk_{rel}"], ii[f"be_{rel}"])
        w = np.zeros((34, 128), np.float32)
        w[0:16, 0:64] = Wq; w[0:16, 64:128] = Wv
        w[16, 0:64] = 2 * We; w[16, 64:128] = We
        w[17, 0:64] = bq + bk + 2 * be; w[17, 64:128] = bv + be
        w[18:34, 0:64] = Wk
        # block-diagonal 3x stack: one matmul transforms 3 subtiles
        w3 = np.zeros((102, 384), np.float32)
        for t in range(3):
            w3[t * 34:(t + 1) * 34, t * 128:(t + 1) * 128] = w
        return w3.astype(EDT)

    def wskip(rel):
        w = np.zeros((17, 64), np.float32)
        w[0:16] = ii[f"Wskip_{rel}"]
        w[16] = ii[f"bconv_{rel}"]
        return w.astype(EDT)

    mlp_w = {
        "W1": ii["W1"].astype(np.float16), "W2": ii["W2"].astype(np.float16),
        "W3": ii["W3"].astype(np.float16), "Wout": ii["Wout"].astype(np.float16),
        "b1": ii["b1"].astype(np.float32).reshape(64, 1),
        "b2": ii["b2"].astype(np.float32).reshape(64, 1),
        "b3": ii["b3"].astype(np.float32).reshape(64, 1),
        "bout": ii["bout"].astype(np.float32).reshape(1, 1),
    }

    # ---------------- device program ----------------
    nc = bacc.Bacc("TRN2", target_bir_lowering=False, debug=False,
                   num_devices=NCORES)

    def din(name, arr0):
        return nc.dram_tensor(name, list(arr0.shape),
                              mybir.dt.from_np(arr0.dtype), kind="ExternalInput")

    h = {}
    for tag, rel in (("c", rel_c), ("b", rel_b)):
        pc0 = rel["per_core"][0]
        for nm in ("xt", "oh", "pa", "og"):
            h[f"{nm}_{tag}"] = din(f"{nm}_{tag}", pc0[nm])
    h["waug_c"] = din("waug_c", waug("ac"))
    h["waug_b"] = din("waug_b", waug("cb"))
    h["wskip_c"] = din("wskip_c", wskip("ac"))
    h["wskip_b"] = din("wskip_b", wskip("cb"))
    h["recip"] = din("recip", recip)
    sel2 = np.zeros((2, 128), np.float16); sel2[0, 0:64] = 1; sel2[1, 64:128] = 1
    h["ones2"] = din("ones2", sel2)
    for k, v in mlp_w.items():
        h["mlp_" + k] = din("mlp_" + k, v)
    out_h = nc.dram_tensor("out", [1, G], F32, kind="ExternalOutput")

    with tile.TileContext(nc) as tc:
        with tc.tile_pool(name="const", bufs=1) as cp, \
             tc.tile_pool(name="acc", bufs=1) as accp, \
             tc.tile_pool(name="stream", bufs=6) as sp, \
             tc.tile_pool(name="work", bufs=4) as wp, \
             tc.tile_pool(name="psum", bufs=2, space="PSUM") as pp, \
             tc.tile_pool(name="psA", bufs=1, space="PSUM") as ppA, \
             tc.tile_pool(name="dram", bufs=1, space="DRAM") as dp:

            pooled_ps = ppA.tile([128, G], F32, tag="pooled_ps")

            def relation(tag, rel, row_off):
                nbuck = rel["nbuck"]
                subtiles = rel["subtiles"]
                ntiles = int(subtiles.sum())
                sub_start = np.zeros(nbuck + 1, np.int64)
                sub_start[1:] = np.cumsum(subtiles)
                first_of = {int(sub_start[b]): b for b in range(nbuck)}
                last_of = {int(sub_start[b + 1]) - 1: b for b in range(nbuck)}

                w_t = cp.tile([102, 384], FE, name=f"waug_{tag}",
                              tag=f"waug_{tag}")
                nc.sync.dma_start(w_t[:], h[f"waug_{tag}"].ap())
                ws_t = cp.tile([17, 64], FE, name=f"wskip_{tag}", tag=f"wskip_{tag}")
                nc.sync.dma_start(ws_t[:], h[f"wskip_{tag}"].ap())

                h_sb = accp.tile([128, nbuck * 64], FE, name=f"h_{tag}",
                                 tag=f"h_{tag}")
                pa_sb = accp.tile([17, nbuck * BUCKET], FE, name=f"pa_{tag}",
                                  tag=f"pa_{tag}")
                nc.sync.dma_start(pa_sb[:], h[f"pa_{tag}"].ap())
                og_sb = accp.tile([128, nbuck * 128], FE, name=f"og_{tag}",
                                  tag=f"og_{tag}")
                nc.sync.dma_start(og_sb[:], h[f"og_{tag}"].ap())

                xt_v = h[f"xt_{tag}"].ap()
                oh_v = h[f"oh_{tag}"].ap()
                ntiles3 = rel["ntiles3"]
                bps = None
                for t0 in range(0, ntiles3, GRP):
                    g = min(GRP, ntiles3 - t0)
                    g3 = g // 3
                    e0 = t0 * 128
                    xt_t = sp.tile([102, (GRP // 3) * 128], FE,
                                   name=f"xt_{tag}_{t0}", tag="xt")
                    nc.sync.dma_start(xt_t[:, :g3 * 128],
                                      xt_v[:, (t0 // 3) * 128:
                                           (t0 // 3 + g3) * 128])
                    oh_t = sp.tile([128, GRP, 128], FE, name=f"oh_{tag}_{t0}",
                                   tag="oh")
                    oh2 = oh_t[:].rearrange("p a b -> p (a b)")
                    nc.sync.dma_start(oh2[:, :g * 128], oh_v[:, e0:e0 + g * 128])
                    # each triple-matmul writes a 512-col (2KB) aligned PSUM
                    # region: cols [q*512, q*512+384) hold subtiles 3q..3q+2
                    sv = pp.tile([128, (GRP // 3) * 512], F32,
                                 name=f"sv_{tag}_{t0}", tag="sv")
                    for q in range(g3):
                        nc.tensor.matmul(sv[:, q * 512:q * 512 + 384],
                                         xt_t[:, q * 128:(q + 1) * 128],
                                         w_t[:], start=True, stop=True)
                    sv4 = sv[:].rearrange("p (q r h) -> p q r h", q=GRP // 3,
                                          r=4)
                    gt = wp.tile([128, GRP, 64], F16, name=f"gt_{tag}_{t0}",
                                 tag="gt")
                    gt4 = gt[:].rearrange("p (q r) h -> p q r h", q=GRP // 3)
                    nc.scalar.activation(gt4[:, :g3, :, :],
                                         sv4[:, :g3, 0:3, 0:64], AF.Sigmoid)
                    msg = wp.tile([128, GRP, 64], FE, name=f"msg_{tag}_{t0}",
                                  tag="msg")
                    msg4 = msg[:].rearrange("p (q r) h -> p q r h", q=GRP // 3)
                    nc.vector.tensor_tensor(msg4[:, :g3, :, :],
                                            gt4[:, :g3, :, :],
                                            sv4[:, :g3, 0:3, 64:128],
                                            op=OP.mult)
                    for j in range(g):
                        t = t0 + j
                        if t >= ntiles:
                            break
                        if t in first_of:
                            b = first_of[t]
                            bps = pp.tile([128, 64], F32, name=f"bps_{tag}_{b}",
                                          tag="bps")
                            nc.tensor.matmul(
                                bps[:], pa_sb[:, b * BUCKET:(b + 1) * BUCKET],
                                ws_t[:], start=True, stop=False,
                                skip_group_check=True)
                        is_last = t in last_of
                        nc.tensor.matmul(bps[:], oh_t[:, j, :], msg[:, j, :],
                                         start=False, stop=is_last,
                                         skip_group_check=True)
                        if is_last:
                            b = last_of[t]
                            nc.vector.tensor_scalar(
                                h_sb[:, b * 64:(b + 1) * 64], bps[:],
                                0.0, None, OP.max)
                            nc.tensor.matmul(pooled_ps[row_off:row_off + 64, :],
                                             h_sb[:, b * 64:(b + 1) * 64],
                                             og_sb[:, b * 128:(b + 1) * 128],
                                             start=(b == 0),
                                             stop=(b == nbuck - 1),
                                             skip_group_check=True)

            relation("c", rel_c, 0)
            relation("b", rel_b, 64)

            pooled_sb = accp.tile([128, G], F32, tag="pooled_sb")
            nc.vector.tensor_copy(pooled_sb[:], pooled_ps[:])
            bounce_in = dp.tile([128, G], F32, tag="bounce_in")
            bounce_out = dp.tile([128, G], F32, tag="bounce_out")
            nc.sync.dma_start(bounce_in[:], pooled_sb[:])
            nc.gpsimd.collective_compute(
                "AllReduce", OP.add, replica_groups=[list(range(NCORES))],
                ins=[bounce_in.opt()], outs=[bounce_out.opt()])
            nc.sync.dma_start(pooled_sb[:], bounce_out[:])

            recip_sb = accp.tile([2, G], F16, tag="recip_sb")
            nc.sync.dma_start(recip_sb[:], h["recip"].ap())
            ones2_sb = accp.tile([2, 128], F16, tag="ones2_sb")
            nc.sync.dma_start(ones2_sb[:], h["ones2"].ap())
            rb_ps = ppA.tile([128, G], F32, tag="mlps")
            nc.tensor.matmul(rb_ps[:], ones2_sb[:], recip_sb[:],
                             start=True, stop=True)
            mean_sb = accp.tile([128, G], F16, tag="mean_sb")
            nc.vector.tensor_tensor(mean_sb[:], pooled_sb[:], rb_ps[:], op=OP.mult)

            mw, mb = {}, {}
            for k in ("W1", "W2", "W3", "Wout"):
                mw[k] = accp.tile(list(mlp_w[k].shape), F16, name=f"mw{k}",
                                  tag=f"mw{k}")
                nc.sync.dma_start(mw[k][:], h["mlp_" + k].ap())
            for k in ("b1", "b2", "b3", "bout"):
                mb[k] = accp.tile(list(mlp_w[k].shape), F32, name=f"mb{k}",
                                  tag=f"mb{k}")
                nc.sync.dma_start(mb[k][:], h["mlp_" + k].ap())

            hcur = mean_sb
            for li, (wk, bk) in enumerate((("W1", "b1"), ("W2", "b2"),
                                           ("W3", "b3"))):
                ps = ppA.tile([64, G], F32, name=f"mlp{li}", tag="mlps")
                nc.tensor.matmul(ps[:], mw[wk][:], hcur[:], start=True, stop=True)
                hn = accp.tile([64, G], F16, name=f"hn{li}", tag=f"hn{li}")
                nc.scalar.activation(hn[:], ps[:], AF.Relu, bias=mb[bk][:])
                hcur = hn
            ps_o = ppA.tile([1, G], F32, tag="mlps")
            nc.tensor.matmul(ps_o[:], mw["Wout"][:], hcur[:], start=True, stop=True)
            osb = accp.tile([1, G], F32, tag="osb")
            nc.scalar.activation(osb[:], ps_o[:], AF.Identity, bias=mb["bout"][:])
            nc.sync.dma_start(out_h.ap(), osb[:])

    nc.compile()

    in_maps = []
    for m in range(NCORES):
        mp = {}
        for tag, rel in (("c", rel_c), ("b", rel_b)):
            pc = rel["per_core"][m]
            for nm in ("xt", "oh", "pa", "og"):
                mp[f"{nm}_{tag}"] = pc[nm]
        mp.update({
            "waug_c": waug("ac"), "waug_b": waug("cb"),
            "wskip_c": wskip("ac"), "wskip_b": wskip("cb"),
            "recip": recip, "ones2": sel2,
            **{"mlp_" + k: v for k, v in mlp_w.items()},
        })
        in_maps.append(mp)
    import os
    trace = bool(os.environ.get("KERNEL_TRACE"))
    res = run_bass_kernel_spmd(nc, in_maps, core_ids=list(range(NCORES)),
                               trace=trace)
    global LAST_EXEC_NS
    LAST_EXEC_NS = res.exec_time_ns
    return res.results[0]["out"].reshape(G).astype(np.float32)


# revision 29
# speedup vs baseline: 1.1640x; 1.1640x over previous
"""Trainium2 Bass kernel for hetero-GNN (2x ResGatedGraphConv + segment-mean pooling + MLP).

v3:
  - Host-streamed fp8 one-hot scatter matrices (no on-device IS_EQ; fp8 FWL LDW).
  - fp8 edge stream / weights / skip inputs; fp16 sigmoid output; fp8 messages.
  - Global dst-buckets sorted by edge count and dealt round-robin to (core, k)
    so the shared SPMD program's per-k subtile count (max over cores) is tight.
  - Sigmoid/multiply batched over GRP=8 subtiles in flat groups; every 3rd
    multiply offloaded to GPSIMD.
  - Skip-connection matmul opens each bucket's PSUM accumulation; ReLU fused
    into the PSUM->SBUF copy; pooling matmul emitted inline per bucket.
"""
import sys
import types
import numpy as np
import ml_dtypes

NCORES = 8
G = 128
H = 64
F = 16
NC_N = 100000
NB_N = 200000
BUCKET = 128
GRP = 6  # subtiles (x128 edges) per streaming group; must be multiple of 3
FP8 = True
GP_EVERY = 0  # gpsimd multiply offload disabled: GPSIMD cannot access PSUM
LAST_EXEC_NS = None

F8NP = np.dtype(ml_dtypes.float8_e4m3)
ONE_F8 = np.float32(1.0).astype(F8NP).view(np.uint8)
EDT = F8NP if FP8 else np.float16


def _install_ntff_shim():
    if 'antenv.axon_hooks' in sys.modules:
        return
    try:
        mod = types.ModuleType('antenv.axon_hooks')
        _h = [None]
        mod.set_axon_ntff_profile_hook = lambda h: _h.__setitem__(0, h)
        mod.get_axon_ntff_profile_hook = lambda: _h[0]
        sys.modules['antenv.axon_hooks'] = mod
        import antenv
        antenv.axon_hooks = mod
        from trn_agent_boot.trn_boot import _ntff_profile_via_ctypes
        mod.set_axon_ntff_profile_hook(
            _ntff_profile_via_ctypes('/opt/axon/libaxon_pjrt.so'))
    except Exception:
        pass


def _prep_relation(x_src, x_dst, src, dst, ea, batch, n_dst):
    """Host marshalling with bucket dealing.

    Global 128-dst buckets are sorted by edge count (desc) and dealt to
    (core, k): core m's k-th bucket is sorted_buckets[8k+m]. All cores share
    the same per-k subtile count = ceil(largest bucket of round k / 128).
    Returns per-core xt/oh streams plus pa (skip features), bt (graph ids).
    """
    nbuck_g = (n_dst + BUCKET - 1) // BUCKET
    nbuck = (nbuck_g + NCORES - 1) // NCORES
    n_slots_g = nbuck * NCORES  # padded with empty buckets

    gb = dst // BUCKET
    cnt = np.bincount(gb, minlength=n_slots_g)
    order_b = np.argsort(-cnt, kind="stable")  # bucket ids, desc by count
    # round k takes sorted buckets [8k, 8k+8); per-k subtiles from the max
    subtiles = np.maximum(
        (cnt[order_b[::NCORES]] + 127) // 128, 1)  # [nbuck]
    ntiles = int(subtiles.sum())
    ntiles3 = ((ntiles + 2) // 3) * 3  # transform packs 3 subtiles per matmul
    ntot = ntiles3 * 128
    starts = np.zeros(nbuck + 1, np.int64)
    starts[1:] = np.cumsum(subtiles) * 128

    # bucket -> (core, k)
    core_of_b = np.empty(n_slots_g, np.int64)
    k_of_b = np.empty(n_slots_g, np.int64)
    core_of_b[order_b] = np.arange(n_slots_g) % NCORES
    k_of_b[order_b] = np.arange(n_slots_g) // NCORES

    edge_core = core_of_b[gb]
    edge_k = k_of_b[gb]
    xs = x_src.astype(EDT)
    xd = x_dst.astype(EDT)
    batf = batch.astype(np.float32)

    per_core = []
    for m in range(NCORES):
        sel = np.nonzero(edge_core == m)[0]
        # sort this core's edges by k (stable)
        sel = sel[np.argsort(edge_k[sel], kind="stable")]
        c_src, c_dst, c_ea = src[sel], dst[sel], ea[sel, 0]
        c_k = edge_k[sel]
        pos_in_bucket = np.arange(len(sel)) - np.searchsorted(c_k, c_k)
        slot = starts[c_k] + pos_in_bucket
        xt = np.zeros((34, ntot), EDT)
        xt[0:16, slot] = xs[c_src].T
        xt[16, slot] = c_ea.astype(EDT)
        xt[17, slot] = np.float32(1.0)
        xt[18:34, slot] = xd[c_dst].T
        # stack 3 consecutive subtiles along K: xt3[:, t*128+m] holds the
        # 34-row features of subtiles 3t, 3t+1, 3t+2 at slot m
        xtr = xt.reshape(34, ntiles3, 128)
        xt3 = np.concatenate([xtr[:, 0::3], xtr[:, 1::3], xtr[:, 2::3]],
                             axis=0).reshape(102, (ntiles3 // 3) * 128)
        xt3 = np.ascontiguousarray(xt3)
        oh = np.zeros((128, ntot), np.uint8)
        d_loc = c_dst % BUCKET
        oh[slot % 128, (slot // 128) * 128 + d_loc] = ONE_F8
        # skip features + streamed pooling one-hot for this core's buckets
        pa = np.zeros((17, nbuck * BUCKET), EDT)
        og = np.zeros((128, nbuck * 128), np.uint8)
        my_b = order_b[np.arange(nbuck) * NCORES + m]  # global bucket per k
        for k in range(nbuck):
            b0 = my_b[k] * BUCKET
            w = min(BUCKET, n_dst - b0)
            if w <= 0:
                continue
            pa[0:16, k * BUCKET:k * BUCKET + w] = xd[b0:b0 + w].T
            pa[16, k * BUCKET:k * BUCKET + w] = np.float32(1.0)
            gids = batf[b0:b0 + w].astype(np.int64)
            og[np.arange(w), k * 128 + gids] = ONE_F8
        per_core.append({"xt": xt3, "oh": oh.view(F8NP), "pa": pa,
                         "og": og.view(F8NP)})
    return {"nbuck": nbuck, "subtiles": subtiles, "ntot": ntot,
            "ntiles3": ntiles3, "per_core": per_core}


def kernel(**inputs):
    _install_ntff_shim()
    import concourse.bass as bass  # noqa: F401
    import concourse.bacc as bacc
    import concourse.mybir as mybir
    import concourse.tile as tile
    from concourse.bass_utils import run_bass_kernel_spmd

    F32 = mybir.dt.float32
    F16 = mybir.dt.float16
    FE = mybir.dt.float8e4 if FP8 else mybir.dt.float16
    AF = mybir.ActivationFunctionType
    OP = mybir.AluOpType

    ii = {k: np.asarray(v) for k, v in inputs.items()}

    rel_c = _prep_relation(ii["x_x"], ii["x_c"], ii["src_ac"].astype(np.int64),
                           ii["dst_ac"].astype(np.int64), ii["ea_ac"],
                           ii["batch_c"].astype(np.int64), NC_N)
    rel_b = _prep_relation(ii["x_c"], ii["x_b"], ii["src_cb"].astype(np.int64),
                           ii["dst_cb"].astype(np.int64), ii["ea_cb"],
                           ii["batch_b"].astype(np.int64), NB_N)

    cnt_c = np.bincount(ii["batch_c"].astype(np.int64), minlength=G).astype(np.float32)
    cnt_b = np.bincount(ii["batch_b"].astype(np.int64), minlength=G).astype(np.float32)
    recip = np.stack([1.0 / np.maximum(cnt_c, 1.0),
                      1.0 / np.maximum(cnt_b, 1.0)]).astype(np.float16)  # [2, G]

    def waug(rel):
        Wq, Wv, Wk = ii[f"Wq_{rel}"], ii[f"Wv_{rel}"], ii[f"Wk_{rel}"]
        We = ii[f"We_{rel}"][0]
        bq, bv, bk, be = (ii[f"bq_{rel}"], ii[f"bv_{rel}"],
                          ii[f"bk_{rel}"], ii[f"be_{rel}"])
        w = np.zeros((34, 128), np.float32)
        w[0:16, 0:64] = Wq; w[0:16, 64:128] = Wv
        w[16, 0:64] = 2 * We; w[16, 64:128] = We
        w[17, 0:64] = bq + bk + 2 * be; w[17, 64:128] = bv + be
        w[18:34, 0:64] = Wk
        # block-diagonal 3x stack: one matmul transforms 3 subtiles.
        # bf16 moving operand streams 2 cols/cycle on the PE.
        w3 = np.zeros((102, 384), np.float32)
        for t in range(3):
            w3[t * 34:(t + 1) * 34, t * 128:(t + 1) * 128] = w
        return w3.astype(ml_dtypes.bfloat16)

    def wskip(rel):
        w = np.zeros((17, 64), np.float32)
        w[0:16] = ii[f"Wskip_{rel}"]
        w[16] = ii[f"bconv_{rel}"]
        return w.astype(EDT)

    mlp_w = {
        "W1": ii["W1"].astype(np.float16), "W2": ii["W2"].astype(np.float16),
        "W3": ii["W3"].astype(np.float16), "Wout": ii["Wout"].astype(np.float16),
        "b1": ii["b1"].astype(np.float32).reshape(64, 1),
        "b2": ii["b2"].astype(np.float32).reshape(64, 1),
        "b3": ii["b3"].astype(np.float32).reshape(64, 1),
        "bout": ii["bout"].astype(np.float32).reshape(1, 1),
    }

    # ---------------- device program ----------------
    nc = bacc.Bacc("TRN2", target_bir_lowering=False, debug=False,
                   num_devices=NCORES)

    def din(name, arr0):
        return nc.dram_tensor(name, list(arr0.shape),
                              mybir.dt.from_np(arr0.dtype), kind="ExternalInput")

    h = {}
    for tag, rel in (("c", rel_c), ("b", rel_b)):
        pc0 = rel["per_core"][0]
        for nm in ("xt", "oh", "pa", "og"):
            h[f"{nm}_{tag}"] = din(f"{nm}_{tag}", pc0[nm])
    h["waug_c"] = din("waug_c", waug("ac"))
    h["waug_b"] = din("waug_b", waug("cb"))
    h["wskip_c"] = din("wskip_c", wskip("ac"))
    h["wskip_b"] = din("wskip_b", wskip("cb"))
    h["recip"] = din("recip", recip)
    sel2 = np.zeros((2, 128), np.float16); sel2[0, 0:64] = 1; sel2[1, 64:128] = 1
    h["ones2"] = din("ones2", sel2)
    for k, v in mlp_w.items():
        h["mlp_" + k] = din("mlp_" + k, v)
    out_h = nc.dram_tensor("out", [1, G], F32, kind="ExternalOutput")

    with tile.TileContext(nc) as tc:
        with tc.tile_pool(name="const", bufs=1) as cp, \
             tc.tile_pool(name="acc", bufs=1) as accp, \
             tc.tile_pool(name="stream", bufs=6) as sp, \
             tc.tile_pool(name="work", bufs=4) as wp, \
             tc.tile_pool(name="psum", bufs=2, space="PSUM") as pp, \
             tc.tile_pool(name="psA", bufs=1, space="PSUM") as ppA, \
             tc.tile_pool(name="dram", bufs=1, space="DRAM") as dp:

            pooled_ps = ppA.tile([128, G], F32, tag="pooled_ps")

            def relation(tag, rel, row_off):
                nbuck = rel["nbuck"]
                subtiles = rel["subtiles"]
                ntiles = int(subtiles.sum())
                sub_start = np.zeros(nbuck + 1, np.int64)
                sub_start[1:] = np.cumsum(subtiles)
                first_of = {int(sub_start[b]): b for b in range(nbuck)}
                last_of = {int(sub_start[b + 1]) - 1: b for b in range(nbuck)}

                w_t = cp.tile([102, 384], mybir.dt.bfloat16,
                              name=f"waug_{tag}", tag=f"waug_{tag}")
                nc.sync.dma_start(w_t[:], h[f"waug_{tag}"].ap())
                ws_t = cp.tile([17, 64], FE, name=f"wskip_{tag}", tag=f"wskip_{tag}")
                nc.sync.dma_start(ws_t[:], h[f"wskip_{tag}"].ap())

                h_sb = accp.tile([128, nbuck * 64], FE, name=f"h_{tag}",
                                 tag=f"h_{tag}")
                pa_sb = accp.tile([17, nbuck * BUCKET], FE, name=f"pa_{tag}",
                                  tag=f"pa_{tag}")
                nc.sync.dma_start(pa_sb[:], h[f"pa_{tag}"].ap())
                og_sb = accp.tile([128, nbuck * 128], FE, name=f"og_{tag}",
                                  tag=f"og_{tag}")
                nc.sync.dma_start(og_sb[:], h[f"og_{tag}"].ap())

                xt_v = h[f"xt_{tag}"].ap()
                oh_v = h[f"oh_{tag}"].ap()
                ntiles3 = rel["ntiles3"]
                bps = None
                for t0 in range(0, ntiles3, GRP):
                    g = min(GRP, ntiles3 - t0)
                    g3 = g // 3
                    e0 = t0 * 128
                    xt_t = sp.tile([102, (GRP // 3) * 128], FE,
                                   name=f"xt_{tag}_{t0}", tag="xt")
                    nc.sync.dma_start(xt_t[:, :g3 * 128],
                                      xt_v[:, (t0 // 3) * 128:
                                           (t0 // 3 + g3) * 128])
                    oh_t = sp.tile([128, GRP, 128], FE, name=f"oh_{tag}_{t0}",
                                   tag="oh")
                    oh2 = oh_t[:].rearrange("p a b -> p (a b)")
                    nc.sync.dma_start(oh2[:, :g * 128], oh_v[:, e0:e0 + g * 128])
                    # each triple-matmul writes a 512-col (2KB) aligned PSUM
                    # region: cols [q*512, q*512+384) hold subtiles 3q..3q+2
                    sv = pp.tile([128, (GRP // 3) * 512], F32,
                                 name=f"sv_{tag}_{t0}", tag="sv")
                    for q in range(g3):
                        nc.tensor.matmul(sv[:, q * 512:q * 512 + 384],
                                         xt_t[:, q * 128:(q + 1) * 128],
                                         w_t[:], start=True, stop=True)
                    sv4 = sv[:].rearrange("p (q r h) -> p q r h", q=GRP // 3,
                                          r=4)
                    gt = wp.tile([128, GRP, 64], F16, name=f"gt_{tag}_{t0}",
                                 tag="gt")
                    gt4 = gt[:].rearrange("p (q r) h -> p q r h", q=GRP // 3)
                    nc.scalar.activation(gt4[:, :g3, :, :],
                                         sv4[:, :g3, 0:3, 0:64], AF.Sigmoid)
                    msg = wp.tile([128, GRP, 64], FE, name=f"msg_{tag}_{t0}",
                                  tag="msg")
                    msg4 = msg[:].rearrange("p (q r) h -> p q r h", q=GRP // 3)
                    nc.vector.tensor_tensor(msg4[:, :g3, :, :],
                                            gt4[:, :g3, :, :],
                                            sv4[:, :g3, 0:3, 64:128],
                                            op=OP.mult)
                    for j in range(g):
                        t = t0 + j
                        if t >= ntiles:
                            break
                        if t in first_of:
                            b = first_of[t]
                            bps = pp.tile([128, 64], F32, name=f"bps_{tag}_{b}",
                                          tag="bps")
                            nc.tensor.matmul(
                                bps[:], pa_sb[:, b * BUCKET:(b + 1) * BUCKET],
                                ws_t[:], start=True, stop=False,
                                skip_group_check=True)
                        is_last = t in last_of
                        nc.tensor.matmul(bps[:], oh_t[:, j, :], msg[:, j, :],
                                         start=False, stop=is_last,
                                         skip_group_check=True)
                        if is_last:
                            b = last_of[t]
                            nc.vector.tensor_scalar(
                                h_sb[:, b * 64:(b + 1) * 64], bps[:],
                                0.0, None, OP.max)
                            nc.tensor.matmul(pooled_ps[row_off:row_off + 64, :],
                                             h_sb[:, b * 64:(b + 1) * 64],
                                             og_sb[:, b * 128:(b + 1) * 128],
                                             start=(b == 0),
                                             stop=(b == nbuck - 1),
                                             skip_group_check=True)

            relation("c", rel_c, 0)
            relation("b", rel_b, 64)

            pooled_sb = accp.tile([128, G], F32, tag="pooled_sb")
            nc.vector.tensor_copy(pooled_sb[:], pooled_ps[:])
            bounce_in = dp.tile([128, G], F32, tag="bounce_in")
            bounce_out = dp.tile([128, G], F32, tag="bounce_out")
            nc.sync.dma_start(bounce_in[:], pooled_sb[:])
            nc.gpsimd.collective_compute(
                "AllReduce", OP.add, replica_groups=[list(range(NCORES))],
                ins=[bounce_in.opt()], outs=[bounce_out.opt()])
            nc.sync.dma_start(pooled_sb[:], bounce_out[:])

            recip_sb = accp.tile([2, G], F16, tag="recip_sb")
            nc.sync.dma_start(recip_sb[:], h["recip"].ap())
            ones2_sb = accp.tile([2, 128], F16, tag="ones2_sb")
            nc.sync.dma_start(ones2_sb[:], h["ones2"].ap())
            rb_ps = ppA.tile([128, G], F32, tag="mlps")
            nc.tensor.matmul(rb_ps[:], ones2_sb[:], recip_sb[:],
                             start=True, stop=True)
            mean_sb = accp.tile([128, G], F16, tag="mean_sb")
            nc.vector.tensor_tensor(mean_sb[:], pooled_sb[:], rb_ps[:], op=OP.mult)

            mw, mb = {}, {}
            for k in ("W1", "W2", "W3", "Wout"):
                mw[k] = accp.tile(list(mlp_w[k].shape), F16, name=f"mw{k}",
                                  tag=f"mw{k}")
                nc.sync.dma_start(mw[k][:], h["mlp_" + k].ap())
            for k in ("b1", "b2", "b3", "bout"):
                mb[k] = accp.tile(list(mlp_w[k].shape), F32, name=f"mb{k}",
                                  tag=f"mb{k}")
                nc.sync.dma_start(mb[k][:], h["mlp_" + k].ap())

            hcur = mean_sb
            for li, (wk, bk) in enumerate((("W1", "b1"), ("W2", "b2"),
                                           ("W3", "b3"))):
                ps = ppA.tile([64, G], F32, name=f"mlp{li}", tag="mlps")
                nc.tensor.matmul(ps[:], mw[wk][:], hcur[:], start=True, stop=True)
                hn = accp.tile([64, G], F16, name=f"hn{li}", tag=f"hn{li}")
                nc.scalar.activation(hn[:], ps[:], AF.Relu, bias=mb[bk][:])
                hcur = hn
            ps_o = ppA.tile([1, G], F32, tag="mlps")
            nc.tensor.matmul(ps_o[:], mw["Wout"][:], hcur[:], start=True, stop=True)
            osb = accp.tile([1, G], F32, tag="osb")
            nc.scalar.activation(osb[:], ps_o[:], AF.Identity, bias=mb["bout"][:])
            nc.sync.dma_start(out_h.ap(), osb[:])

    nc.compile()

    in_maps = []
    for m in range(NCORES):
        mp = {}
        for tag, rel in (("c", rel_c), ("b", rel_b)):
            pc = rel["per_core"][m]
            for nm in ("xt", "oh", "pa", "og"):
                mp[f"{nm}_{tag}"] = pc[nm]
        mp.update({
            "waug_c": waug("ac"), "waug_b": waug("cb"),
            "wskip_c": wskip("ac"), "wskip_b": wskip("cb"),
            "recip": recip, "ones2": sel2,
            **{"mlp_" + k: v for k, v in mlp_w.items()},
        })
        in_maps.append(mp)
    import os
    trace = bool(os.environ.get("KERNEL_TRACE"))
    res = run_bass_kernel_spmd(nc, in_maps, core_ids=list(range(NCORES)),
                               trace=trace)
    global LAST_EXEC_NS
    LAST_EXEC_NS = res.exec_time_ns
    return res.results[0]["out"].reshape(G).astype(np.float32)


# revision 31
# speedup vs baseline: 1.1659x; 1.0017x over previous
"""Trainium2 Bass kernel for hetero-GNN (2x ResGatedGraphConv + segment-mean pooling + MLP).

v3:
  - Host-streamed fp8 one-hot scatter matrices (no on-device IS_EQ; fp8 FWL LDW).
  - fp8 edge stream / weights / skip inputs; fp16 sigmoid output; fp8 messages.
  - Global dst-buckets sorted by edge count and dealt round-robin to (core, k)
    so the shared SPMD program's per-k subtile count (max over cores) is tight.
  - Sigmoid/multiply batched over GRP=8 subtiles in flat groups; every 3rd
    multiply offloaded to GPSIMD.
  - Skip-connection matmul opens each bucket's PSUM accumulation; ReLU fused
    into the PSUM->SBUF copy; pooling matmul emitted inline per bucket.
"""
import sys
import types
import numpy as np
import ml_dtypes

NCORES = 8
G = 128
H = 64
F = 16
NC_N = 100000
NB_N = 200000
BUCKET = 128
GRP = 6  # subtiles (x128 edges) per streaming group; must be multiple of 3
FP8 = True
GP_EVERY = 0  # gpsimd multiply offload disabled: GPSIMD cannot access PSUM
LAST_EXEC_NS = None

F8NP = np.dtype(ml_dtypes.float8_e4m3)
ONE_F8 = np.float32(1.0).astype(F8NP).view(np.uint8)
EDT = F8NP if FP8 else np.float16


def _install_ntff_shim():
    if 'antenv.axon_hooks' in sys.modules:
        return
    try:
        mod = types.ModuleType('antenv.axon_hooks')
        _h = [None]
        mod.set_axon_ntff_profile_hook = lambda h: _h.__setitem__(0, h)
        mod.get_axon_ntff_profile_hook = lambda: _h[0]
        sys.modules['antenv.axon_hooks'] = mod
        import antenv
        antenv.axon_hooks = mod
        from trn_agent_boot.trn_boot import _ntff_profile_via_ctypes
        mod.set_axon_ntff_profile_hook(
            _ntff_profile_via_ctypes('/opt/axon/libaxon_pjrt.so'))
    except Exception:
        pass


def _prep_relation(x_src, x_dst, src, dst, ea, batch, n_dst):
    """Host marshalling with bucket dealing.

    Global 128-dst buckets are sorted by edge count (desc) and dealt to
    (core, k): core m's k-th bucket is sorted_buckets[8k+m]. All cores share
    the same per-k subtile count = ceil(largest bucket of round k / 128).
    Returns per-core xt/oh streams plus pa (skip features), bt (graph ids).
    """
    nbuck_g = (n_dst + BUCKET - 1) // BUCKET
    nbuck = (nbuck_g + NCORES - 1) // NCORES
    n_slots_g = nbuck * NCORES  # padded with empty buckets

    gb = dst // BUCKET
    cnt = np.bincount(gb, minlength=n_slots_g)
    order_b = np.argsort(-cnt, kind="stable")  # bucket ids, desc by count
    # round k takes sorted buckets [8k, 8k+8); per-k subtiles from the max
    subtiles = np.maximum(
        (cnt[order_b[::NCORES]] + 127) // 128, 1)  # [nbuck]
    ntiles = int(subtiles.sum())
    ntiles3 = ((ntiles + 2) // 3) * 3  # transform packs 3 subtiles per matmul
    ntot = ntiles3 * 128
    starts = np.zeros(nbuck + 1, np.int64)
    starts[1:] = np.cumsum(subtiles) * 128

    # bucket -> (core, k)
    core_of_b = np.empty(n_slots_g, np.int64)
    k_of_b = np.empty(n_slots_g, np.int64)
    core_of_b[order_b] = np.arange(n_slots_g) % NCORES
    k_of_b[order_b] = np.arange(n_slots_g) // NCORES

    edge_core = core_of_b[gb]
    edge_k = k_of_b[gb]
    xs = x_src.astype(EDT)
    xd = x_dst.astype(EDT)
    batf = batch.astype(np.float32)

    per_core = []
    for m in range(NCORES):
        sel = np.nonzero(edge_core == m)[0]
        # sort this core's edges by k (stable)
        sel = sel[np.argsort(edge_k[sel], kind="stable")]
        c_src, c_dst, c_ea = src[sel], dst[sel], ea[sel, 0]
        c_k = edge_k[sel]
        pos_in_bucket = np.arange(len(sel)) - np.searchsorted(c_k, c_k)
        slot = starts[c_k] + pos_in_bucket
        xt = np.zeros((34, ntot), EDT)
        xt[0:16, slot] = xs[c_src].T
        xt[16, slot] = c_ea.astype(EDT)
        xt[17, slot] = np.float32(1.0)
        xt[18:34, slot] = xd[c_dst].T
        # stack 3 consecutive subtiles along K: xt3[:, t*128+m] holds the
        # 34-row features of subtiles 3t, 3t+1, 3t+2 at slot m
        xtr = xt.reshape(34, ntiles3, 128)
        xt3 = np.concatenate([xtr[:, 0::3], xtr[:, 1::3], xtr[:, 2::3]],
                             axis=0).reshape(102, (ntiles3 // 3) * 128)
        xt3 = np.ascontiguousarray(xt3)
        oh = np.zeros((128, ntot), np.uint8)
        d_loc = c_dst % BUCKET
        oh[slot % 128, (slot // 128) * 128 + d_loc] = ONE_F8
        # skip features + streamed pooling one-hot for this core's buckets
        pa = np.zeros((17, nbuck * BUCKET), EDT)
        og = np.zeros((128, nbuck * 128), np.uint8)
        my_b = order_b[np.arange(nbuck) * NCORES + m]  # global bucket per k
        for k in range(nbuck):
            b0 = my_b[k] * BUCKET
            w = min(BUCKET, n_dst - b0)
            if w <= 0:
                continue
            pa[0:16, k * BUCKET:k * BUCKET + w] = xd[b0:b0 + w].T
            pa[16, k * BUCKET:k * BUCKET + w] = np.float32(1.0)
            gids = batf[b0:b0 + w].astype(np.int64)
            og[np.arange(w), k * 128 + gids] = ONE_F8
        per_core.append({"xt": xt3, "oh": oh.view(F8NP), "pa": pa,
                         "og": og.view(F8NP)})
    return {"nbuck": nbuck, "subtiles": subtiles, "ntot": ntot,
            "ntiles3": ntiles3, "per_core": per_core}


def kernel(**inputs):
    _install_ntff_shim()
    import concourse.bass as bass  # noqa: F401
    import concourse.bacc as bacc
    import concourse.mybir as mybir
    import concourse.tile as tile
    from concourse.bass_utils import run_bass_kernel_spmd

    F32 = mybir.dt.float32
    F16 = mybir.dt.float16
    FE = mybir.dt.float8e4 if FP8 else mybir.dt.float16
    AF = mybir.ActivationFunctionType
    OP = mybir.AluOpType

    ii = {k: np.asarray(v) for k, v in inputs.items()}

    rel_c = _prep_relation(ii["x_x"], ii["x_c"], ii["src_ac"].astype(np.int64),
                           ii["dst_ac"].astype(np.int64), ii["ea_ac"],
                           ii["batch_c"].astype(np.int64), NC_N)
    rel_b = _prep_relation(ii["x_c"], ii["x_b"], ii["src_cb"].astype(np.int64),
                           ii["dst_cb"].astype(np.int64), ii["ea_cb"],
                           ii["batch_b"].astype(np.int64), NB_N)

    cnt_c = np.bincount(ii["batch_c"].astype(np.int64), minlength=G).astype(np.float32)
    cnt_b = np.bincount(ii["batch_b"].astype(np.int64), minlength=G).astype(np.float32)
    recip = np.stack([1.0 / np.maximum(cnt_c, 1.0),
                      1.0 / np.maximum(cnt_b, 1.0)]).astype(np.float16)  # [2, G]

    def waug(rel):
        Wq, Wv, Wk = ii[f"Wq_{rel}"], ii[f"Wv_{rel}"], ii[f"Wk_{rel}"]
        We = ii[f"We_{rel}"][0]
        bq, bv, bk, be = (ii[f"bq_{rel}"], ii[f"bv_{rel}"],
                          ii[f"bk_{rel}"], ii[f"be_{rel}"])
        w = np.zeros((34, 128), np.float32)
        w[0:16, 0:64] = Wq; w[0:16, 64:128] = Wv
        w[16, 0:64] = 2 * We; w[16, 64:128] = We
        w[17, 0:64] = bq + bk + 2 * be; w[17, 64:128] = bv + be
        w[18:34, 0:64] = Wk
        # block-diagonal 3x stack: one matmul transforms 3 subtiles.
        # bf16 moving operand streams 2 cols/cycle on the PE.
        w3 = np.zeros((102, 384), np.float32)
        for t in range(3):
            w3[t * 34:(t + 1) * 34, t * 128:(t + 1) * 128] = w
        return w3.astype(ml_dtypes.bfloat16)

    def wskip(rel):
        w = np.zeros((17, 64), np.float32)
        w[0:16] = ii[f"Wskip_{rel}"]
        w[16] = ii[f"bconv_{rel}"]
        return w.astype(EDT)

    mlp_w = {
        "W1": ii["W1"].astype(np.float16), "W2": ii["W2"].astype(np.float16),
        "W3": ii["W3"].astype(np.float16), "Wout": ii["Wout"].astype(np.float16),
        "b1": ii["b1"].astype(np.float32).reshape(64, 1),
        "b2": ii["b2"].astype(np.float32).reshape(64, 1),
        "b3": ii["b3"].astype(np.float32).reshape(64, 1),
        "bout": ii["bout"].astype(np.float32).reshape(1, 1),
    }

    # ---------------- device program ----------------
    nc = bacc.Bacc("TRN2", target_bir_lowering=False, debug=False,
                   num_devices=NCORES)

    def din(name, arr0):
        return nc.dram_tensor(name, list(arr0.shape),
                              mybir.dt.from_np(arr0.dtype), kind="ExternalInput")

    h = {}
    for tag, rel in (("c", rel_c), ("b", rel_b)):
        pc0 = rel["per_core"][0]
        for nm in ("xt", "oh", "pa", "og"):
            h[f"{nm}_{tag}"] = din(f"{nm}_{tag}", pc0[nm])
    h["waug_c"] = din("waug_c", waug("ac"))
    h["waug_b"] = din("waug_b", waug("cb"))
    h["wskip_c"] = din("wskip_c", wskip("ac"))
    h["wskip_b"] = din("wskip_b", wskip("cb"))
    h["recip"] = din("recip", recip)
    sel2 = np.zeros((2, 128), np.float16); sel2[0, 0:64] = 1; sel2[1, 64:128] = 1
    h["ones2"] = din("ones2", sel2)
    for k, v in mlp_w.items():
        h["mlp_" + k] = din("mlp_" + k, v)
    out_h = nc.dram_tensor("out", [1, G], F32, kind="ExternalOutput")

    with tile.TileContext(nc) as tc:
        with tc.tile_pool(name="const", bufs=1) as cp, \
             tc.tile_pool(name="acc", bufs=1) as accp, \
             tc.tile_pool(name="stream", bufs=8) as sp, \
             tc.tile_pool(name="work", bufs=6) as wp, \
             tc.tile_pool(name="psum", bufs=2, space="PSUM") as pp, \
             tc.tile_pool(name="psA", bufs=1, space="PSUM") as ppA, \
             tc.tile_pool(name="dram", bufs=1, space="DRAM") as dp:

            pooled_ps = ppA.tile([128, G], F32, tag="pooled_ps")

            def relation(tag, rel, row_off):
                nbuck = rel["nbuck"]
                subtiles = rel["subtiles"]
                ntiles = int(subtiles.sum())
                sub_start = np.zeros(nbuck + 1, np.int64)
                sub_start[1:] = np.cumsum(subtiles)
                first_of = {int(sub_start[b]): b for b in range(nbuck)}
                last_of = {int(sub_start[b + 1]) - 1: b for b in range(nbuck)}

                w_t = cp.tile([102, 384], mybir.dt.bfloat16,
                              name=f"waug_{tag}", tag=f"waug_{tag}")
                nc.sync.dma_start(w_t[:], h[f"waug_{tag}"].ap())
                ws_t = cp.tile([17, 64], FE, name=f"wskip_{tag}", tag=f"wskip_{tag}")
                nc.sync.dma_start(ws_t[:], h[f"wskip_{tag}"].ap())

                h_sb = accp.tile([128, nbuck * 64], FE, name=f"h_{tag}",
                                 tag=f"h_{tag}")
                pa_sb = accp.tile([17, nbuck * BUCKET], FE, name=f"pa_{tag}",
                                  tag=f"pa_{tag}")
                nc.sync.dma_start(pa_sb[:], h[f"pa_{tag}"].ap())
                og_sb = accp.tile([128, nbuck * 128], FE, name=f"og_{tag}",
                                  tag=f"og_{tag}")
                nc.sync.dma_start(og_sb[:], h[f"og_{tag}"].ap())

                xt_v = h[f"xt_{tag}"].ap()
                oh_v = h[f"oh_{tag}"].ap()
                ntiles3 = rel["ntiles3"]
                bps = None
                for t0 in range(0, ntiles3, GRP):
                    g = min(GRP, ntiles3 - t0)
                    g3 = g // 3
                    e0 = t0 * 128
                    xt_t = sp.tile([102, (GRP // 3) * 128], FE,
                                   name=f"xt_{tag}_{t0}", tag="xt")
                    nc.sync.dma_start(xt_t[:, :g3 * 128],
                                      xt_v[:, (t0 // 3) * 128:
                                           (t0 // 3 + g3) * 128])
                    oh_t = sp.tile([128, GRP, 128], FE, name=f"oh_{tag}_{t0}",
                                   tag="oh")
                    oh2 = oh_t[:].rearrange("p a b -> p (a b)")
                    nc.sync.dma_start(oh2[:, :g * 128], oh_v[:, e0:e0 + g * 128])
                    # each triple-matmul writes a 512-col (2KB) aligned PSUM
                    # region: cols [q*512, q*512+384) hold subtiles 3q..3q+2
                    sv = pp.tile([128, (GRP // 3) * 512], F32,
                                 name=f"sv_{tag}_{t0}", tag="sv")
                    for q in range(g3):
                        nc.tensor.matmul(sv[:, q * 512:q * 512 + 384],
                                         xt_t[:, q * 128:(q + 1) * 128],
                                         w_t[:], start=True, stop=True)
                    sv4 = sv[:].rearrange("p (q r h) -> p q r h", q=GRP // 3,
                                          r=4)
                    gt = wp.tile([128, GRP, 64], F16, name=f"gt_{tag}_{t0}",
                                 tag="gt")
                    gt4 = gt[:].rearrange("p (q r) h -> p q r h", q=GRP // 3)
                    nc.scalar.activation(gt4[:, :g3, :, :],
                                         sv4[:, :g3, 0:3, 0:64], AF.Sigmoid)
                    msg = wp.tile([128, GRP, 64], FE, name=f"msg_{tag}_{t0}",
                                  tag="msg")
                    msg4 = msg[:].rearrange("p (q r) h -> p q r h", q=GRP // 3)
                    nc.vector.tensor_tensor(msg4[:, :g3, :, :],
                                            gt4[:, :g3, :, :],
                                            sv4[:, :g3, 0:3, 64:128],
                                            op=OP.mult)
                    for j in range(g):
                        t = t0 + j
                        if t >= ntiles:
                            break
                        if t in first_of:
                            b = first_of[t]
                            bps = pp.tile([128, 64], F32, name=f"bps_{tag}_{b}",
                                          tag="bps")
                            nc.tensor.matmul(
                                bps[:], pa_sb[:, b * BUCKET:(b + 1) * BUCKET],
                                ws_t[:], start=True, stop=False,
                                skip_group_check=True)
                        is_last = t in last_of
                        nc.tensor.matmul(bps[:], oh_t[:, j, :], msg[:, j, :],
                                         start=False, stop=is_last,
                                         skip_group_check=True)
                        if is_last:
                            b = last_of[t]
                            nc.vector.tensor_scalar(
                                h_sb[:, b * 64:(b + 1) * 64], bps[:],
                                0.0, None, OP.max)
                            nc.tensor.matmul(pooled_ps[row_off:row_off + 64, :],
                                             h_sb[:, b * 64:(b + 1) * 64],
                                             og_sb[:, b * 128:(b + 1) * 128],
                                             start=(b == 0),
                                             stop=(b == nbuck - 1),
                                             skip_group_check=True)

            relation("c", rel_c, 0)
            relation("b", rel_b, 64)

            pooled_h = accp.tile([128, G], mybir.dt.bfloat16, tag="pooled_h")
            nc.vector.tensor_copy(pooled_h[:], pooled_ps[:])
            bounce_in = dp.tile([128, G], mybir.dt.bfloat16, tag="bounce_in")
            bounce_out = dp.tile([128, G], mybir.dt.bfloat16, tag="bounce_out")
            nc.sync.dma_start(bounce_in[:], pooled_h[:])
            nc.gpsimd.collective_compute(
                "AllReduce", OP.add, replica_groups=[list(range(NCORES))],
                ins=[bounce_in.opt()], outs=[bounce_out.opt()])
            pooled_sb = accp.tile([128, G], mybir.dt.bfloat16, tag="pooled_sb")
            nc.sync.dma_start(pooled_sb[:], bounce_out[:])

            recip_sb = accp.tile([2, G], F16, tag="recip_sb")
            nc.sync.dma_start(recip_sb[:], h["recip"].ap())
            ones2_sb = accp.tile([2, 128], F16, tag="ones2_sb")
            nc.sync.dma_start(ones2_sb[:], h["ones2"].ap())
            rb_ps = ppA.tile([128, G], F32, tag="mlps")
            nc.tensor.matmul(rb_ps[:], ones2_sb[:], recip_sb[:],
                             start=True, stop=True)
            mean_sb = accp.tile([128, G], F16, tag="mean_sb")
            nc.vector.tensor_tensor(mean_sb[:], pooled_sb[:], rb_ps[:], op=OP.mult)

            mw, mb = {}, {}
            for k in ("W1", "W2", "W3", "Wout"):
                mw[k] = accp.tile(list(mlp_w[k].shape), F16, name=f"mw{k}",
                                  tag=f"mw{k}")
                nc.sync.dma_start(mw[k][:], h["mlp_" + k].ap())
            for k in ("b1", "b2", "b3", "bout"):
                mb[k] = accp.tile(list(mlp_w[k].shape), F32, name=f"mb{k}",
                                  tag=f"mb{k}")
                nc.sync.dma_start(mb[k][:], h["mlp_" + k].ap())

            hcur = mean_sb
            for li, (wk, bk) in enumerate((("W1", "b1"), ("W2", "b2"),
                                           ("W3", "b3"))):
                ps = ppA.tile([64, G], F32, name=f"mlp{li}", tag="mlps")
                nc.tensor.matmul(ps[:], mw[wk][:], hcur[:], start=True, stop=True)
                hn = accp.tile([64, G], F16, name=f"hn{li}", tag=f"hn{li}")
                nc.scalar.activation(hn[:], ps[:], AF.Relu, bias=mb[bk][:])
                hcur = hn
            ps_o = ppA.tile([1, G], F32, tag="mlps")
            nc.tensor.matmul(ps_o[:], mw["Wout"][:], hcur[:], start=True, stop=True)
            osb = accp.tile([1, G], F32, tag="osb")
            nc.scalar.activation(osb[:], ps_o[:], AF.Identity, bias=mb["bout"][:])
            nc.sync.dma_start(out_h.ap(), osb[:])

    nc.compile()

    in_maps = []
    for m in range(NCORES):
        mp = {}
        for tag, rel in (("c", rel_c), ("b", rel_b)):
            pc = rel["per_core"][m]
            for nm in ("xt", "oh", "pa", "og"):
                mp[f"{nm}_{tag}"] = pc[nm]
        mp.update({
            "waug_c": waug("ac"), "waug_b": waug("cb"),
            "wskip_c": wskip("ac"), "wskip_b": wskip("cb"),
            "recip": recip, "ones2": sel2,
            **{"mlp_" + k: v for k, v in mlp_w.items()},
        })
        in_maps.append(mp)
    import os
    trace = bool(os.environ.get("KERNEL_TRACE"))
    res = run_bass_kernel_spmd(nc, in_maps, core_ids=list(range(NCORES)),
                               trace=trace)
    global LAST_EXEC_NS
    LAST_EXEC_NS = res.exec_time_ns
    return res.results[0]["out"].reshape(G).astype(np.float32)
